# revision 7
# baseline (speedup 1.0000x reference)
"""Trainium2 Bass kernel for BlockDecomposedSSMAttention.

Math: y[b,s,:] = x[b,s,:] @ B.T @ A @ C.T   (no cross-block recurrence)
 ==>  y = x @ W  with  W = B.T @ A @ C.T

Distribution over the 8 NeuronCores (grid = 2 row-groups x 4 col-quarters):
  core c = (rg, cq):  computes y[rg*8192:(rg+1)*8192, cq*256:(cq+1)*256]
  - x rows are split 2 ways (8192 rows/core, read by 4 cores each).
  - Each core only needs W[:, cq*256:(cq+1)*256], so the W-build stages
    shrink 4x vs full-W-per-core:  T = A @ C.T[:, quarter]   (1024x256)
                                   W_q = B.T @ T             (1024x256)
  - Stages run kt-outer so A.T / C.T / B stream in as fully-contiguous
    256 KiB per-k-tile chunks (2 KB descriptor rows = DMA line rate) in
    exactly consumption order.
  - Main loop keeps the moving dim at 512 by making W the stationary
    operand: psum tiles are y.T [128 o' x 512 m]; the host transposes.
  - All matmul operands are bf16 (same 1 cycle/row PE rate as f32r, half
    the HBM/SBUF bytes, fast FWL weight loads); PSUM accumulates fp32.
    y is written bf16 and upcast to fp32 on the host.

Host-side work is layout marshalling (shard slicing, transposes, dtype
casts); every MAC runs on the device.
"""

import os
import sys

import numpy as np

if "/opt/trn_rl_repo" not in sys.path:
    sys.path.insert(0, "/opt/trn_rl_repo")

import ml_dtypes

BF16 = ml_dtypes.bfloat16

BATCH, SEQ, D = 4, 4096, 1024
NCORES = 8
RG, CQ = 2, 4                 # row-groups x col-quarters
ROWS = BATCH * SEQ            # 16384
MSH = ROWS // RG              # 8192 rows per core
OD = D // CQ                  # 256 output cols per core
P = 128
KT = D // P                   # 8 contraction tiles
MC = 512                      # moving chunk of m in the main loop
NMC = MSH // MC               # 16 m-chunks
NOT = OD // P                 # 2 o'-tiles

_CACHE: dict = {}


def _build_nc():
    import concourse.mybir as mybir
    import concourse.tile as tile
    from concourse import bacc

    f32 = mybir.dt.float32
    bf16 = mybir.dt.bfloat16

    nc = bacc.Bacc(
        "TRN2", target_bir_lowering=False, debug=False, num_devices=NCORES
    )

    # Per-core inputs (bf16, contraction dim on partitions, k/j-tile-major
    # so each DMA chunk is one fully-contiguous slab):
    #   at [ko, kp, j]  = A[j, ko*128+kp]            (A.T, replicated)
    #   bt [jo, jp, i]  = B[jo*128+jp, i]            (B,   replicated)
    #   ct [ko, kp, o]  = C[cq*256+o, ko*128+kp]     (C.T col-quarter)
    #   xt [ip, io, m]  = x2[rg*8192+m, io*128+ip]   (x row-shard, transposed)
    at_in = nc.dram_tensor("at_in", [KT, P, D], bf16, kind="ExternalInput")
    b_in = nc.dram_tensor("b_in", [KT, P, D], bf16, kind="ExternalInput")
    ct_in = nc.dram_tensor("ct_in", [KT, P, OD], bf16, kind="ExternalInput")
    xt = nc.dram_tensor("xt", [P, KT, MSH], bf16, kind="ExternalInput")
    # y.T shard [o', m]; host transposes + upcasts.
    y_out = nc.dram_tensor("y_out", [OD, MSH], bf16, kind="ExternalOutput")

    with tile.TileContext(nc) as tc:
        with (
            tc.tile_pool(name="big", bufs=1) as big,
            tc.tile_pool(name="ycopy", bufs=6) as ycopy,
            tc.tile_pool(name="pss", bufs=4, space="PSUM") as pss,
            tc.tile_pool(name="psm", bufs=4, space="PSUM") as psm,
        ):
            at_sb = big.tile([P, KT, D], bf16)
            ct_sb = big.tile([P, KT, OD], bf16)
            bt_sb = big.tile([P, KT, D], bf16)
            t_sb = big.tile([P, KT, OD], bf16)
            w_sb = big.tile([P, KT, OD], bf16)
            xt_sb = big.tile([P, KT, MSH], bf16)

            # ---- input DMAs, in exact consumption order ----
            # stage-1 consumes (ct[kt], at[kt]) pairs, kt ascending
            for kt in range(KT):
                nc.sync.dma_start(ct_sb[:, kt, :], ct_in.ap()[kt])
                nc.sync.dma_start(at_sb[:, kt, :], at_in.ap()[kt])
            # stage-2 consumes bt[jt], jt ascending; the first xt chunks are
            # slotted between the bt halves so main can start right after
            # stage 2 without waiting on x.
            for jt in range(4):
                nc.sync.dma_start(bt_sb[:, jt, :], b_in.ap()[jt])
            for mq in range(2):
                nc.sync.dma_start(
                    xt_sb[:, :, mq * 512 : (mq + 1) * 512],
                    xt.ap()[:, :, mq * 512 : (mq + 1) * 512],
                )
            for jt in range(4, KT):
                nc.sync.dma_start(bt_sb[:, jt, :], b_in.ap()[jt])
            for mq in range(1, 8):
                nc.sync.dma_start(
                    xt_sb[:, :, mq * 1024 : (mq + 1) * 1024],
                    xt.ap()[:, :, mq * 1024 : (mq + 1) * 1024],
                )

            # ---- stage 1: T = A @ Ct_q  [1024 x 256], kt-outer ----
            # two half-passes of 4 live psum banks each, so the main pool's
            # 4 banks are never blocked behind stage copies.
            for half in range(2):
                ps1 = [pss.tile([P, OD], f32, name="pss") for j in range(4)]
                for kt in range(KT):
                    for j4 in range(4):
                        jt = half * 4 + j4
                        nc.tensor.matmul(
                            ps1[j4][:],
                            at_sb[:, kt, jt * P : (jt + 1) * P],
                            ct_sb[:, kt, :],
                            start=(kt == 0),
                            stop=(kt == KT - 1),
                        )
                for j4 in range(4):
                    nc.vector.tensor_copy(
                        t_sb[:, half * 4 + j4, :], ps1[j4][:]
                    )

            # ---- stage 2: W_q = B.T @ T  [1024 x 256], jt-outer ----
            for half in range(2):
                ps2 = [pss.tile([P, OD], f32, name="pss") for i in range(4)]
                for jt in range(KT):
                    for i4 in range(4):
                        it = half * 4 + i4
                        nc.tensor.matmul(
                            ps2[i4][:],
                            bt_sb[:, jt, it * P : (it + 1) * P],
                            t_sb[:, jt, :],
                            start=(jt == 0),
                            stop=(jt == KT - 1),
                        )
                for i4 in range(4):
                    nc.vector.tensor_copy(
                        w_sb[:, half * 4 + i4, :], ps2[i4][:]
                    )

            # ---- main: y_q.T = W_q.T @ x.T  [256 x 8192] ----
            # W stationary (reused across m), x moving at N=512.
            # m-groups of 2 chunks x 2 o'-tiles -> 4 psum banks per group.
            for mg in range(NMC // 2):
                pms = [psm.tile([P, MC], f32, name="psm") for i in range(2 * NOT)]
                for ot in range(NOT):
                    for it in range(KT):
                        for mc in range(2):
                            m0 = (mg * 2 + mc) * MC
                            nc.tensor.matmul(
                                pms[2 * ot + mc][:],
                                w_sb[:, it, ot * P : (ot + 1) * P],
                                xt_sb[:, it, m0 : m0 + MC],
                                start=(it == 0),
                                stop=(it == KT - 1),
                            )
                for ot in range(NOT):
                    for mc in range(2):
                        m0 = (mg * 2 + mc) * MC
                        yt = ycopy.tile([P, MC], bf16, name="yt")
                        nc.vector.tensor_copy(yt[:], pms[2 * ot + mc][:])
                        nc.scalar.dma_start(
                            y_out.ap()[ot * P : (ot + 1) * P, m0 : m0 + MC],
                            yt[:],
                        )

    nc.compile()
    return nc


def _get_nc():
    if "nc" not in _CACHE:
        _CACHE["nc"] = _build_nc()
    return _CACHE["nc"]


def _make_in_maps(x, A, B, C):
    x2 = np.ascontiguousarray(x, dtype=np.float32).reshape(ROWS, D)
    at = np.ascontiguousarray(np.asarray(A, np.float32).T).reshape(
        KT, P, D
    ).astype(BF16)
    bt = np.asarray(B, np.float32).reshape(KT, P, D).astype(BF16)
    xts = []
    for rg in range(RG):
        shard = x2[rg * MSH : (rg + 1) * MSH]  # [MSH, D]
        xts.append(
            np.ascontiguousarray(
                shard.reshape(MSH, KT, P).transpose(2, 1, 0)
            ).astype(BF16)
        )
    in_maps = []
    for c in range(NCORES):
        rg, cq = divmod(c, CQ)
        csl = np.asarray(C, np.float32)[cq * OD : (cq + 1) * OD, :]  # [OD, D]
        ct = np.ascontiguousarray(csl.T).reshape(KT, P, OD).astype(BF16)
        in_maps.append({"at_in": at, "b_in": bt, "ct_in": ct, "xt": xts[rg]})
    return in_maps


def _install_ntff_hook():
    """The agent image's ``antenv`` lacks ``axon_hooks``; recreate it and
    register the ctypes-based NTFF profile hook (same as trn_boot's
    ``_ntff_profile_via_ctypes``) so ``trace=True`` yields exec_time_ns."""
    import contextlib
    import ctypes
    import types

    if "antenv.axon_hooks" in sys.modules:
        return True
    so_path = "/opt/axon/libaxon_pjrt.so"
    if not os.path.exists(so_path):
        return False
    lib = ctypes.CDLL(so_path)
    if not hasattr(lib, "axon_start_nrt_profile"):
        return False
    lib.axon_start_nrt_profile.argtypes = [
        ctypes.POINTER(ctypes.c_int64),
        ctypes.c_size_t,
    ]
    lib.axon_start_nrt_profile.restype = ctypes.c_int64
    lib.axon_stop_nrt_profile.argtypes = [ctypes.c_char_p]
    lib.axon_stop_nrt_profile.restype = ctypes.c_int64

    @contextlib.contextmanager
    def _hook(output_dir, device_ids):
        import jax

        jax.devices()
        if device_ids:
            ids = (ctypes.c_int64 * len(device_ids))(*device_ids)
            rc = lib.axon_start_nrt_profile(ids, len(device_ids))
        else:
            rc = lib.axon_start_nrt_profile(None, 0)
        if rc != 0:
            raise RuntimeError(f"axon_start_nrt_profile rc={rc}")
        try:
            yield
        finally:
            n = lib.axon_stop_nrt_profile(str(output_dir).encode())
            print(f"ntff profile: {n} file(s) written to {output_dir}")

    mod = types.ModuleType("antenv.axon_hooks")
    _state = {"hook": _hook}
    mod.set_axon_ntff_profile_hook = lambda h: _state.__setitem__("hook", h)
    mod.get_axon_ntff_profile_hook = lambda: _state["hook"]
    sys.modules["antenv.axon_hooks"] = mod
    import antenv

    antenv.axon_hooks = mod
    return True


def run(x, A, B, C, trace=False):
    """Run on hardware; returns (y_full, exec_time_ns_or_None)."""
    from concourse import bass_utils
    from concourse.bass_interp import get_hw_module

    if trace and not _install_ntff_hook():
        trace = False
    if trace:
        # upload_artifacts pushes the NEFF dir to a remote bucket; in this
        # sandbox that can fail AFTER a successful run, losing the results.
        # Degrade to the local path. (Only touches the tracing dev path.)
        if not getattr(bass_utils.upload_artifacts, "_safe", False):
            _orig_upload = bass_utils.upload_artifacts

            def _safe_upload(tmpdir):
                try:
                    return _orig_upload(tmpdir)
                except Exception as e:
                    print(f"upload_artifacts skipped ({type(e).__name__}): {e}")
                    return str(tmpdir)

            _safe_upload._safe = True
            bass_utils.upload_artifacts = _safe_upload

    nc = _get_nc()
    in_maps = _make_in_maps(x, A, B, C)

    old_m = nc.m
    nc.m = get_hw_module(nc.m)
    try:
        res = bass_utils.run_bass_kernel_spmd(
            nc, in_maps, core_ids=list(range(NCORES)), trace=trace
        )
    finally:
        nc.m = old_m

    y2 = np.empty((ROWS, D), dtype=np.float32)
    for c in range(NCORES):
        rg, cq = divmod(c, CQ)
        y2[rg * MSH : (rg + 1) * MSH, cq * OD : (cq + 1) * OD] = (
            res.results[c]["y_out"].T.astype(np.float32)
        )
    return y2.reshape(BATCH, SEQ, D), res.exec_time_ns


def kernel(x, A, B, C):
    y, _ = run(x, A, B, C, trace=False)
    return y


# revision 10
# speedup vs baseline: 1.0732x; 1.0732x over previous
"""Trainium2 Bass kernel for BlockDecomposedSSMAttention.

Math: y[b,s,:] = x[b,s,:] @ B.T @ A @ C.T   (no cross-block recurrence)
 ==>  y = x @ W  with  W = B.T @ A @ C.T

Distribution over the 8 NeuronCores (grid = 2 row-groups x 4 col-quarters):
  core c = (rg, cq):  computes y[rg*8192:(rg+1)*8192, cq*256:(cq+1)*256]
  - x rows are split 2 ways (8192 rows/core, read by 4 cores each).
  - Each core only needs W[:, cq*256:(cq+1)*256], so the W-build stages
    shrink 4x vs full-W-per-core:  T = A @ C.T[:, quarter]   (1024x256)
                                   W_q = B.T @ T             (1024x256)
  - Stages run kt-outer so A.T / C.T / B stream in as fully-contiguous
    256 KiB per-k-tile chunks (2 KB descriptor rows = DMA line rate) in
    exactly consumption order.
  - Main loop keeps the moving dim at 512 by making W the stationary
    operand: psum tiles are y.T [128 o' x 512 m]; the host transposes.
  - All matmul operands are bf16 (same 1 cycle/row PE rate as f32r, half
    the HBM/SBUF bytes, fast FWL weight loads); PSUM accumulates fp32.
    y is written bf16 and upcast to fp32 on the host.

Host-side work is layout marshalling (shard slicing, transposes, dtype
casts); every MAC runs on the device.
"""

import os
import sys

import numpy as np

if "/opt/trn_rl_repo" not in sys.path:
    sys.path.insert(0, "/opt/trn_rl_repo")

import ml_dtypes

BF16 = ml_dtypes.bfloat16

BATCH, SEQ, D = 4, 4096, 1024
NCORES = 8
RG, CQ = 2, 4                 # row-groups x col-quarters
ROWS = BATCH * SEQ            # 16384
MSH = ROWS // RG              # 8192 rows per core
OD = D // CQ                  # 256 output cols per core
P = 128
KT = D // P                   # 8 contraction tiles
MC = 512                      # moving chunk of m in the main loop
NMC = MSH // MC               # 16 m-chunks
NOT = OD // P                 # 2 o'-tiles

_CACHE: dict = {}


def _build_nc():
    import concourse.mybir as mybir
    import concourse.tile as tile
    from concourse import bacc

    f32 = mybir.dt.float32
    bf16 = mybir.dt.bfloat16

    nc = bacc.Bacc(
        "TRN2", target_bir_lowering=False, debug=False, num_devices=NCORES
    )

    # Per-core inputs (bf16, contraction dim on partitions; per-(kp,ko)
    # rows are 512..2048 B contiguous runs, at/above the DMA line-rate
    # threshold):
    #   at [kp, ko, j]  = A[j, ko*128+kp]            (A.T, replicated)
    #   bt [jp, jo, i]  = B[jo*128+jp, i]            (B,   replicated)
    #   ct [kp, ko, o]  = C[cq*256+o, ko*128+kp]     (C.T col-quarter)
    #   xt [ip, io, m]  = x2[rg*8192+m, io*128+ip]   (x row-shard, transposed)
    at_in = nc.dram_tensor("at_in", [P, KT, D], bf16, kind="ExternalInput")
    b_in = nc.dram_tensor("b_in", [P, KT, D], bf16, kind="ExternalInput")
    ct_in = nc.dram_tensor("ct_in", [P, KT, OD], bf16, kind="ExternalInput")
    xt = nc.dram_tensor("xt", [P, KT, MSH], bf16, kind="ExternalInput")
    # y.T shard [o', m]; host transposes + upcasts.
    y_out = nc.dram_tensor("y_out", [OD, MSH], bf16, kind="ExternalOutput")

    with tile.TileContext(nc) as tc:
        with (
            tc.tile_pool(name="big", bufs=1) as big,
            tc.tile_pool(name="ycopy", bufs=8) as ycopy,
            tc.tile_pool(name="pss", bufs=4, space="PSUM") as pss,
            tc.tile_pool(name="psm", bufs=4, space="PSUM") as psm,
        ):
            at_sb = big.tile([P, KT, D], bf16)
            ct_sb = big.tile([P, KT, OD], bf16)
            bt_sb = big.tile([P, KT, D], bf16)
            t_sb = big.tile([P, KT, OD], bf16)
            w_sb = big.tile([P, KT, OD], bf16)
            xt_sb = big.tile([P, KT, MSH], bf16)

            # ---- input DMAs, coalesced (HWDGE descriptor-gen costs ~0.6us
            # per dma_start, serialized on the Sync engine — few big DMAs
            # issue much faster than many small ones), in consumption order.
            nc.sync.dma_start(ct_sb[:, 0:4, :], ct_in.ap()[:, 0:4, :])
            nc.sync.dma_start(at_sb[:, 0:2, :], at_in.ap()[:, 0:2, :])
            nc.sync.dma_start(at_sb[:, 2:4, :], at_in.ap()[:, 2:4, :])
            nc.sync.dma_start(ct_sb[:, 4:8, :], ct_in.ap()[:, 4:8, :])
            nc.sync.dma_start(at_sb[:, 4:6, :], at_in.ap()[:, 4:6, :])
            nc.sync.dma_start(at_sb[:, 6:8, :], at_in.ap()[:, 6:8, :])
            nc.sync.dma_start(bt_sb[:, 0:4, :], b_in.ap()[:, 0:4, :])
            nc.sync.dma_start(bt_sb[:, 4:8, :], b_in.ap()[:, 4:8, :])
            # first main m-chunk early (2 x 1 MiB), rest as 2 MiB chunks
            for mq in range(2):
                nc.sync.dma_start(
                    xt_sb[:, :, mq * 512 : (mq + 1) * 512],
                    xt.ap()[:, :, mq * 512 : (mq + 1) * 512],
                )
            for mq in range(1, 8):
                nc.sync.dma_start(
                    xt_sb[:, :, mq * 1024 : (mq + 1) * 1024],
                    xt.ap()[:, :, mq * 1024 : (mq + 1) * 1024],
                )

            # ---- stage 1: T = A @ Ct_q  [1024 x 256], kt-outer ----
            # two half-passes of 4 live psum banks each, so the main pool's
            # 4 banks are never blocked behind stage copies.
            for half in range(2):
                ps1 = [pss.tile([P, OD], f32, name="pss") for j in range(4)]
                for kt in range(KT):
                    for j4 in range(4):
                        jt = half * 4 + j4
                        nc.tensor.matmul(
                            ps1[j4][:],
                            at_sb[:, kt, jt * P : (jt + 1) * P],
                            ct_sb[:, kt, :],
                            start=(kt == 0),
                            stop=(kt == KT - 1),
                        )
                for j4 in range(4):
                    nc.vector.tensor_copy(
                        t_sb[:, half * 4 + j4, :], ps1[j4][:]
                    )

            # ---- stage 2: W_q = B.T @ T  [1024 x 256], jt-outer ----
            for half in range(2):
                ps2 = [pss.tile([P, OD], f32, name="pss") for i in range(4)]
                for jt in range(KT):
                    for i4 in range(4):
                        it = half * 4 + i4
                        nc.tensor.matmul(
                            ps2[i4][:],
                            bt_sb[:, jt, it * P : (it + 1) * P],
                            t_sb[:, jt, :],
                            start=(jt == 0),
                            stop=(jt == KT - 1),
                        )
                for i4 in range(4):
                    nc.vector.tensor_copy(
                        w_sb[:, half * 4 + i4, :], ps2[i4][:]
                    )

            # ---- main: y_q.T = W_q.T @ x.T  [256 x 8192] ----
            # W stationary (reused across m), x moving at N=512.
            # m-groups of 2 chunks x 2 o'-tiles -> 4 psum banks per group.
            for mg in range(NMC // 2):
                pms = [psm.tile([P, MC], f32, name="psm") for i in range(2 * NOT)]
                for ot in range(NOT):
                    for it in range(KT):
                        for mc in range(2):
                            m0 = (mg * 2 + mc) * MC
                            nc.tensor.matmul(
                                pms[2 * ot + mc][:],
                                w_sb[:, it, ot * P : (ot + 1) * P],
                                xt_sb[:, it, m0 : m0 + MC],
                                start=(it == 0),
                                stop=(it == KT - 1),
                            )
                for ot in range(NOT):
                    for mc in range(2):
                        m0 = (mg * 2 + mc) * MC
                        yt = ycopy.tile([P, MC], bf16, name="yt")
                        nc.vector.tensor_copy(yt[:], pms[2 * ot + mc][:])
                        nc.scalar.dma_start(
                            y_out.ap()[ot * P : (ot + 1) * P, m0 : m0 + MC],
                            yt[:],
                        )

    nc.compile()
    return nc


def _get_nc():
    if "nc" not in _CACHE:
        _CACHE["nc"] = _build_nc()
    return _CACHE["nc"]


def _make_in_maps(x, A, B, C):
    x2 = np.ascontiguousarray(x, dtype=np.float32).reshape(ROWS, D)
    at = np.ascontiguousarray(
        np.asarray(A, np.float32).reshape(D, KT, P).transpose(2, 1, 0)
    ).astype(BF16)
    bt = np.ascontiguousarray(
        np.asarray(B, np.float32).reshape(KT, P, D).transpose(1, 0, 2)
    ).astype(BF16)
    xts = []
    for rg in range(RG):
        shard = x2[rg * MSH : (rg + 1) * MSH]  # [MSH, D]
        xts.append(
            np.ascontiguousarray(
                shard.reshape(MSH, KT, P).transpose(2, 1, 0)
            ).astype(BF16)
        )
    in_maps = []
    for c in range(NCORES):
        rg, cq = divmod(c, CQ)
        csl = np.asarray(C, np.float32)[cq * OD : (cq + 1) * OD, :]  # [OD, D]
        ct = np.ascontiguousarray(
            csl.T.reshape(KT, P, OD).transpose(1, 0, 2)
        ).astype(BF16)
        in_maps.append({"at_in": at, "b_in": bt, "ct_in": ct, "xt": xts[rg]})
    return in_maps


def _install_ntff_hook():
    """The agent image's ``antenv`` lacks ``axon_hooks``; recreate it and
    register the ctypes-based NTFF profile hook (same as trn_boot's
    ``_ntff_profile_via_ctypes``) so ``trace=True`` yields exec_time_ns."""
    import contextlib
    import ctypes
    import types

    if "antenv.axon_hooks" in sys.modules:
        return True
    so_path = "/opt/axon/libaxon_pjrt.so"
    if not os.path.exists(so_path):
        return False
    lib = ctypes.CDLL(so_path)
    if not hasattr(lib, "axon_start_nrt_profile"):
        return False
    lib.axon_start_nrt_profile.argtypes = [
        ctypes.POINTER(ctypes.c_int64),
        ctypes.c_size_t,
    ]
    lib.axon_start_nrt_profile.restype = ctypes.c_int64
    lib.axon_stop_nrt_profile.argtypes = [ctypes.c_char_p]
    lib.axon_stop_nrt_profile.restype = ctypes.c_int64

    @contextlib.contextmanager
    def _hook(output_dir, device_ids):
        import jax

        jax.devices()
        if device_ids:
            ids = (ctypes.c_int64 * len(device_ids))(*device_ids)
            rc = lib.axon_start_nrt_profile(ids, len(device_ids))
        else:
            rc = lib.axon_start_nrt_profile(None, 0)
        if rc != 0:
            raise RuntimeError(f"axon_start_nrt_profile rc={rc}")
        try:
            yield
        finally:
            n = lib.axon_stop_nrt_profile(str(output_dir).encode())
            print(f"ntff profile: {n} file(s) written to {output_dir}")

    mod = types.ModuleType("antenv.axon_hooks")
    _state = {"hook": _hook}
    mod.set_axon_ntff_profile_hook = lambda h: _state.__setitem__("hook", h)
    mod.get_axon_ntff_profile_hook = lambda: _state["hook"]
    sys.modules["antenv.axon_hooks"] = mod
    import antenv

    antenv.axon_hooks = mod
    return True


def run(x, A, B, C, trace=False):
    """Run on hardware; returns (y_full, exec_time_ns_or_None)."""
    from concourse import bass_utils
    from concourse.bass_interp import get_hw_module

    if trace and not _install_ntff_hook():
        trace = False
    if trace:
        # upload_artifacts pushes the NEFF dir to a remote bucket; in this
        # sandbox that can fail AFTER a successful run, losing the results.
        # Degrade to the local path. (Only touches the tracing dev path.)
        if not getattr(bass_utils.upload_artifacts, "_safe", False):
            _orig_upload = bass_utils.upload_artifacts

            def _safe_upload(tmpdir):
                try:
                    return _orig_upload(tmpdir)
                except Exception as e:
                    print(f"upload_artifacts skipped ({type(e).__name__}): {e}")
                    return str(tmpdir)

            _safe_upload._safe = True
            bass_utils.upload_artifacts = _safe_upload

    nc = _get_nc()
    in_maps = _make_in_maps(x, A, B, C)

    old_m = nc.m
    nc.m = get_hw_module(nc.m)
    try:
        res = bass_utils.run_bass_kernel_spmd(
            nc, in_maps, core_ids=list(range(NCORES)), trace=trace
        )
    finally:
        nc.m = old_m

    y2 = np.empty((ROWS, D), dtype=np.float32)
    for c in range(NCORES):
        rg, cq = divmod(c, CQ)
        y2[rg * MSH : (rg + 1) * MSH, cq * OD : (cq + 1) * OD] = (
            res.results[c]["y_out"].T.astype(np.float32)
        )
    return y2.reshape(BATCH, SEQ, D), res.exec_time_ns


def kernel(x, A, B, C):
    y, _ = run(x, A, B, C, trace=False)
    return y
